# revision 3
# baseline (speedup 1.0000x reference)
"""Trainium2 Bass kernel for nn_EquiGroupSamplingIco.

Data-parallel over batch (32 -> 8 cores x 4). Per core:
  hT   = per-channel linear of trajectory            [10, (c,bt)=1024]
  sigT = relu(D_inT.T @ hT)                          [4096, 1024]   (A)
  h2T  = D_outw.T-chunks @ sigT                      [455, 1024]    (B)
  X_l  = braid(h2T)  [(u,c), (m,bt)]    per l        (braid1, via DRAM)
  stage_l = W2C_l.T @ X_l  [(v,g), (m,bt)]           (C)
  trajT_l = braid(stage_l) [(v,m), (g,bt)]           (braid2, via DRAM)
  out_trajT = sum_l icoT_l.T @ trajT_l               [60, (g,bt)=4096]  (D)
  out_xgT   = icoT.T @ x.T (PE-transposed x)         [60, (b,h,c)=2048]
All matmuls in float32r (fp32 data, ~fp22 multiply, fp32 accumulate).
"""
import math
import os
import sys

for _p in ("/opt/trn_rl_repo", "/opt/pypackages"):
    if _p not in sys.path:
        sys.path.append(_p)

import numpy as np

LMAX = 6
DS = [2 * l + 1 for l in range(LMAX + 1)]
OFF = [0]
for _d in DS:
    OFF.append(OFF[-1] + _d * _d)  # block offsets in 455
J455 = OFF[-1]
NCORES = 8
B_LOC = 32 // NCORES
R = B_LOC * 16            # bt rows per core = 64
NCOL = 16 * R             # (c,bt) cols = 1024
G = 4096
BHC = B_LOC * 4 * 128     # x rows per core = 2048
NQ = 4                    # col quarters
QW = NCOL // NQ           # 256

_CACHE = {}


def _host_prep(inputs):
    w1s = np.asarray(inputs["w1s"], np.float32)
    w1v = np.asarray(inputs["w1v"], np.float32)
    A = np.zeros((16, 10, 10), np.float32)
    A[:, 0, 9] = w1s[0, :, 0, 0]
    s3 = 1.0 / math.sqrt(3.0)
    for v in range(3):
        for m in range(3):
            for u in range(3):
                A[:, 1 + 3 * v + m, 3 * u + m] = w1v[0, :, u, v] * s3
    w1pt = np.ascontiguousarray(np.transpose(A, (2, 0, 1)).reshape(10, 160))
    d_inT = np.ascontiguousarray(np.asarray(inputs["D_in"], np.float32).T)  # (10,4096)
    # packed for PE row-groups: k-tile k at rows 32*(k%4)..+10, cols 128*(k//4)..+128
    dinp = np.zeros((128, 1024), np.float32)
    for k in range(32):
        dinp[32 * (k % 4) : 32 * (k % 4) + 10, 128 * (k // 4) : 128 * (k // 4) + 128] = \
            d_inT[:, k * 128 : (k + 1) * 128]
    icoT = np.ascontiguousarray(np.asarray(inputs["ico_wigners"], np.float32).T)  # (455,60)
    w2c = {}
    for l in range(LMAX + 1):
        d = DS[l]
        w2 = np.asarray(inputs[f"w2_{l}"], np.float32)  # (c,g,u,v)
        w2c[l] = np.ascontiguousarray(
            np.transpose(w2, (2, 0, 3, 1)).reshape(16 * d, 64 * d) / math.sqrt(16 * d)
        )
    return w1pt, dinp, icoT, w2c


def _vm_tiles(l):
    """k-tiles over the (v,m)=d^2 rows of block l: list of (row0, nrows)."""
    d2 = DS[l] * DS[l]
    out = []
    r0 = 0
    while r0 < d2:
        out.append((r0, min(128, d2 - r0)))
        r0 += 128
    return out


def _uc_tiles(l):
    """k-tiles over the (u,c)=16d rows of X_l: list of (u_start, n_u) with 16*n_u<=128."""
    d = DS[l]
    out = []
    u0 = 0
    while u0 < d:
        nu = min(8, d - u0)
        out.append((u0, nu))
        u0 += nu
    return out


def _n_chunks(l):
    """N chunking of (m,bt)=64d into >=256-sized pieces (or one small piece)."""
    n = 64 * DS[l]
    if n <= 512:
        return [(0, n)]
    h = n // 2
    return [(0, h), (h, n - h)]


def _build(trace_sim=False):
    import concourse.bacc as bacc
    import concourse.mybir as mybir
    import concourse.tile as tile

    F32R = mybir.dt.float32r
    F32 = mybir.dt.float32
    RELU = mybir.ActivationFunctionType.Relu
    from concourse.masks import make_identity

    nc = bacc.Bacc("TRN2", target_bir_lowering=False, debug=False)

    x_in = nc.dram_tensor("x_in", (BHC, J455), F32R, kind="ExternalInput").ap()
    trajT_in = nc.dram_tensor("trajT_in", (10, R), F32R, kind="ExternalInput").ap()
    w1pt_in = nc.dram_tensor("w1pt_in", (10, 160), F32R, kind="ExternalInput").ap()
    dinp_in = nc.dram_tensor("dinp_in", (128, 1024), F32R, kind="ExternalInput").ap()
    doutw_in = nc.dram_tensor("doutw_in", (G, J455), F32R, kind="ExternalInput").ap()
    icoT_in = nc.dram_tensor("icoT_in", (J455, 60), F32R, kind="ExternalInput").ap()
    w2c_in = {
        l: nc.dram_tensor(f"w2c{l}_in", (16 * DS[l], 64 * DS[l]), F32R, kind="ExternalInput").ap()
        for l in range(LMAX + 1)
    }
    ident_in = nc.dram_tensor("ident_in", (128, 128), F32R, kind="ExternalInput").ap()
    out_xgT = nc.dram_tensor("out_xgT", (60, BHC), F32, kind="ExternalOutput").ap()
    out_trajT = nc.dram_tensor("out_trajT", (60, 64 * R), F32, kind="ExternalOutput").ap()
    scr1 = nc.dram_tensor("scr1", (J455, NCOL), F32R).ap()
    scr2 = nc.dram_tensor("scr2", (J455, 64 * R), F32R).ap()

    JC = [(0, 128), (128, 128), (256, 128), (384, 71)]  # j-chunks of 455
    JT = [(0, 128), (128, 128), (256, 128), (384, 71)]  # j-tiles for x path

    with tile.TileContext(nc, trace_sim=trace_sim) as tc:
        with (
            tc.tile_pool(name="const", bufs=1) as cpool,
            tc.tile_pool(name="sig", bufs=1) as sigpool,
            tc.tile_pool(name="work", bufs=1) as wpool,
            tc.tile_pool(name="ps", bufs=1, space="PSUM") as pspool,
        ):
            # ---------- constants ----------
            w1pt_sb = cpool.tile([10, 160], F32R, name="w1pt_sb")
            nc.sync.dma_start(w1pt_sb[:], w1pt_in[:])
            dinp_sb = cpool.tile([128, 1024], F32R, name="dinp_sb")
            nc.sync.dma_start(dinp_sb[:], dinp_in[:])
            trajT_sb = cpool.tile([10, R], F32R, name="trajT_sb")
            nc.sync.dma_start(trajT_sb[:], trajT_in[:])
            doutw_sb = cpool.tile([128, 32 * J455], F32R, name="doutw_sb")
            for k in range(32):
                nc.sync.dma_start(
                    doutw_sb[:, k * J455 : (k + 1) * J455],
                    doutw_in[k * 128 : (k + 1) * 128, :],
                )
            ident = cpool.tile([128, 128], F32R, name="ident")
            nc.sync.dma_start(ident[:], ident_in[:])
            icoX_sb = {}
            for jt, (j0, jn) in enumerate(JT):
                t = cpool.tile([128, 60], F32R, name=f"icoX{jt}", tag=f"icoX{jt}")
                nc.sync.dma_start(t[0:jn, :], icoT_in[j0 : j0 + jn, :])
                icoX_sb[jt] = t
            icoD_sb = {}
            for l in range(LMAX + 1):
                for ki, (r0, nr) in enumerate(_vm_tiles(l)):
                    t = cpool.tile([128, 60], F32R, name=f"icoD{l}_{ki}", tag=f"icoD{l}_{ki}")
                    nc.sync.dma_start(t[0:nr, :], icoT_in[OFF[l] + r0 : OFF[l] + r0 + nr, :])
                    icoD_sb[(l, ki)] = t

            # ---------- hT ----------
            hT_sb = cpool.tile([128, NCOL], F32R, name="hT_sb")
            for half in range(2):
                ph = pspool.tile([128, 512], F32, name=f"ps_h{half}", tag="ps", bufs=6)
                for cc in range(8):
                    c = half * 8 + cc
                    nc.tensor.matmul(
                        ph[0:10, cc * R : (cc + 1) * R],
                        w1pt_sb[:, c * 10 : (c + 1) * 10],
                        trajT_sb[:],
                        start=True, stop=True,
                    )
                nc.vector.tensor_copy(
                    hT_sb[0:10, half * 512 : (half + 1) * 512], ph[0:10, :]
                )
            for base in (32, 64, 96):
                nc.sync.dma_start(
                    hT_sb[base : base + 10, :],
                    hT_sb[0:10, :],
                )

            # ---------- x path (independent; emitted early for DMA overlap) ----------
            def emit_x_chunk(ch):
                xT = {}
                for jt, (j0, jn) in enumerate(JT):
                    xT[jt] = wpool.tile([128, 512], F32R, name=f"xT{ch}_{jt}", tag=f"xT{jt}", bufs=2)
                for i in range(4):
                    xn = wpool.tile([128, J455], F32R, name=f"xn{ch}_{i}", tag="xnat", bufs=4)
                    nc.sync.dma_start(xn[:], x_in[ch * 512 + i * 128 : ch * 512 + (i + 1) * 128, :])
                    for jt, (j0, jn) in enumerate(JT):
                        pt = pspool.tile([128, 128], F32R, name=f"pt{ch}_{i}_{jt}", tag="psT", bufs=2)
                        nc.tensor.transpose(pt[0:jn, :], xn[:, j0 : j0 + jn], ident[:])
                        nc.vector.tensor_copy(
                            xT[jt][0:jn, i * 128 : (i + 1) * 128],
                            pt[0:jn, :],
                        )
                px = pspool.tile([128, 512], F32, name=f"px{ch}", tag="ps", bufs=6)
                for jt, (j0, jn) in enumerate(JT):
                    nc.tensor.matmul(
                        px[0:60, :], icoX_sb[jt][0:jn, :], xT[jt][0:jn, :],
                        start=(jt == 0), stop=(jt == 3),
                    )
                xo = wpool.tile([60, 512], F32, name=f"xo{ch}", tag="ost", bufs=2)
                nc.scalar.activation(xo[:], px[0:60, :], mybir.ActivationFunctionType.Copy)
                nc.sync.dma_start(out_xgT[:, ch * 512 : (ch + 1) * 512], xo[:])

            # ---------- A + B per quarter ----------
            sig_sb = {}

            def emit_quarter(q):
                q0 = q * QW
                for k in range(32):
                    base = 32 * (k % 4)
                    pa = pspool.tile([128, 512], F32, name=f"pa{q}_{k}", tag="ps", bufs=6)
                    nc.tensor.matmul(
                        pa[:, 0:QW],
                        dinp_sb[base : base + 10, 128 * (k // 4) : 128 * (k // 4) + 128],
                        hT_sb[base : base + 10, q0 : q0 + QW],
                        start=True, stop=True,
                        tile_position=(base, 0),
                    )
                    sg = sigpool.tile([128, QW], F32R, name=f"sig{q}_{k}", tag=f"sig{k}", bufs=1)
                    if k % 2 == 0:
                        nc.scalar.activation(sg[:], pa[:, 0:QW], RELU)
                    else:
                        nc.vector.tensor_scalar_max(sg[:], pa[:, 0:QW], 0.0)
                    sig_sb[k] = sg
                for jc, (j0, jn) in enumerate(JC):
                    pb = pspool.tile([128, 512], F32, name=f"pb{q}_{jc}", tag="ps", bufs=6)
                    for k in range(32):
                        nc.tensor.matmul(
                            pb[0:jn, 0:QW],
                            doutw_sb[:, k * J455 + j0 : k * J455 + j0 + jn],
                            sig_sb[k][:],
                            start=(k == 0), stop=(k == 31),
                        )
                    h2s = wpool.tile([128, QW], F32R, name=f"h2s{q}_{jc}", tag="h2st", bufs=3)
                    nc.vector.tensor_copy(h2s[0:jn, :], pb[0:jn, 0:QW])
                    nc.sync.dma_start(scr1[j0 : j0 + jn, q0 : q0 + QW], h2s[0:jn, :])

            emit_x_chunk(0)
            emit_quarter(0)
            emit_x_chunk(1)
            emit_quarter(1)
            emit_x_chunk(2)
            emit_quarter(2)
            emit_x_chunk(3)
            emit_quarter(3)

            # ---------- braid1 + C + braid2 per l (large l first) ----------
            for l in range(LMAX, -1, -1):
                d = DS[l]
                uc = _uc_tiles(l)
                # braid1: X_l k-tiles [(u,c), (m,bt)]
                X = {}
                for ki, (u0, nu) in enumerate(uc):
                    xt = wpool.tile([128, 64 * 13], F32R, name=f"X{l}_{ki}", tag="X", bufs=3)
                    X[ki] = xt
                    for uu in range(nu):
                        u = u0 + uu
                        src = scr1[OFF[l] + u * d : OFF[l] + (u + 1) * d, :].rearrange(
                            "m (c bt) -> c m bt", c=16
                        )
                        nc.sync.dma_start(
                            xt[uu * 16 : (uu + 1) * 16, 0 : 64 * d], src
                        )
                # C: stage tiles [(v,g) chunks, (m,bt)]
                w2 = {}
                for ki, (u0, nu) in enumerate(uc):
                    wt = wpool.tile([128, 64 * 13], F32R, name=f"w2c{l}_{ki}", tag="w2c", bufs=3)
                    nc.sync.dma_start(
                        wt[0 : 16 * nu, 0 : 64 * d], w2c_in[l][16 * u0 : 16 * (u0 + nu), :]
                    )
                    w2[ki] = wt
                mtot = 64 * d
                stages = []
                mc0 = 0
                while mc0 < mtot:
                    msz = min(128, mtot - mc0)
                    st = wpool.tile([128, 64 * 13], F32R, name=f"st{l}_{mc0}", tag="stage", bufs=8)
                    pc = pspool.tile([128, 512], F32, name=f"pc{l}_{mc0}", tag="ps", bufs=6)
                    for (n0, nn) in _n_chunks(l):
                        for ki in range(len(uc)):
                            nk = 16 * uc[ki][1]
                            nc.tensor.matmul(
                                pc[0:msz, 0:nn],
                                w2[ki][0:nk, mc0 : mc0 + msz],
                                X[ki][0:nk, n0 : n0 + nn],
                                start=(ki == 0), stop=(ki == len(uc) - 1),
                            )
                        nc.vector.tensor_copy(st[0:msz, n0 : n0 + nn], pc[0:msz, 0:nn])
                    stages.append((mc0, msz, st))
                    mc0 += msz
                # braid2: write scr2 rows per v
                for v in range(d):
                    mc0, msz, st = stages[(v * 64) // 128]
                    loc = v * 64 - mc0
                    dst = scr2[OFF[l] + v * d : OFF[l] + (v + 1) * d, :].rearrange(
                        "m (g bt) -> g m bt", g=64
                    )
                    nc.sync.dma_start(dst, st[loc : loc + 64, 0 : 64 * d])

            # ---------- D ----------
            for nch in range(8):
                pd = pspool.tile([128, 512], F32, name=f"pd{nch}", tag="ps", bufs=6)
                groups = [(l, ki, r0, nr) for l in range(LMAX + 1) for ki, (r0, nr) in enumerate(_vm_tiles(l))]
                for gi, (l, ki, r0, nr) in enumerate(groups):
                    tt = wpool.tile([128, 512], F32R, name=f"tt{nch}_{gi}", tag="tT", bufs=3)
                    nc.sync.dma_start(
                        tt[0:nr, :],
                        scr2[OFF[l] + r0 : OFF[l] + r0 + nr, nch * 512 : (nch + 1) * 512],
                    )
                    nc.tensor.matmul(
                        pd[0:60, :], icoD_sb[(l, ki)][0:nr, :], tt[0:nr, :],
                        start=(gi == 0), stop=(gi == len(groups) - 1),
                    )
                to = wpool.tile([60, 512], F32, name=f"to{nch}", tag="ost", bufs=2)
                nc.scalar.activation(to[:], pd[0:60, :], mybir.ActivationFunctionType.Copy)
                nc.sync.dma_start(out_trajT[:, nch * 512 : (nch + 1) * 512], to[:])

    nc.compile()
    return nc


def _get_nc():
    if "nc" not in _CACHE:
        _CACHE["nc"] = _build()
    return _CACHE["nc"]


def make_in_maps(inputs):
    w1pt, dinp, icoT, w2c = _host_prep(inputs)
    x = np.asarray(inputs["x"], np.float32)
    traj = np.asarray(inputs["trajectory"], np.float32)
    doutw = np.ascontiguousarray(np.asarray(inputs["D_outw"], np.float32))
    shared = {
        "w1pt_in": w1pt,
        "dinp_in": dinp,
        "doutw_in": doutw,
        "icoT_in": icoT,
        "ident_in": np.eye(128, dtype=np.float32),
    }
    for l in range(LMAX + 1):
        shared[f"w2c{l}_in"] = w2c[l]
    in_maps = []
    for i in range(NCORES):
        xs = np.ascontiguousarray(x[i * B_LOC : (i + 1) * B_LOC].reshape(BHC, J455))
        ts = np.ascontiguousarray(traj[i * B_LOC : (i + 1) * B_LOC].reshape(R, 10).T)
        in_maps.append(dict(shared, x_in=xs, trajT_in=ts))
    return in_maps


def assemble(results):
    xg = np.zeros((32, 4, 128, 60), np.float32)
    trj = np.zeros((32, 16, 64, 60), np.float32)
    for i, res in enumerate(results):
        xg[i * B_LOC : (i + 1) * B_LOC] = (
            res["out_xgT"].T.reshape(B_LOC, 4, 128, 60)
        )
        trj[i * B_LOC : (i + 1) * B_LOC] = (
            res["out_trajT"].reshape(60, 64, B_LOC, 16).transpose(2, 3, 1, 0)
        )
    return xg, trj


def kernel(**inputs):
    from concourse.bass_utils import run_bass_kernel_spmd

    nc = _get_nc()
    in_maps = make_in_maps(inputs)
    trace = bool(int(os.environ.get("KERNEL_TRACE", "0")))
    res = run_bass_kernel_spmd(nc, in_maps, core_ids=list(range(NCORES)), trace=trace)
    _CACHE["last_result"] = res
    return assemble(res.results)


# revision 5
# speedup vs baseline: 685.8145x; 685.8145x over previous
"""Trainium2 Bass kernel for nn_EquiGroupSamplingIco.

Data-parallel over batch (32 -> 8 cores x 4). Per core:
  hT   = per-channel linear of trajectory            [10, (c,bt)=1024]
  sigT = relu(D_inT.T @ hT)                          [4096, 1024]   (A)
  h2T  = D_outw.T-chunks @ sigT                      [455, 1024]    (B)
  X_l  = braid(h2T)  [(u,c), (m,bt)]    per l        (braid1, via DRAM)
  stage_l = W2C_l.T @ X_l  [(v,g), (m,bt)]           (C)
  trajT_l = braid(stage_l) [(v,m), (g,bt)]           (braid2, via DRAM)
  out_trajT = sum_l icoT_l.T @ trajT_l               [60, (g,bt)=4096]  (D)
  out_xgT   = icoT.T @ x.T (PE-transposed x)         [60, (b,h,c)=2048]
All matmuls in float32r (fp32 data, ~fp22 multiply, fp32 accumulate).
"""
import math
import os
import sys

for _p in ("/opt/trn_rl_repo", "/opt/pypackages"):
    if _p not in sys.path:
        sys.path.append(_p)

import numpy as np

LMAX = 6
DS = [2 * l + 1 for l in range(LMAX + 1)]
OFF = [0]
for _d in DS:
    OFF.append(OFF[-1] + _d * _d)  # block offsets in 455
J455 = OFF[-1]
NCORES = 8
B_LOC = 32 // NCORES
R = B_LOC * 16            # bt rows per core = 64
NCOL = 16 * R             # (c,bt) cols = 1024
G = 4096
BHC = B_LOC * 4 * 128     # x rows per core = 2048
NQ = 4                    # col quarters
QW = NCOL // NQ           # 256

_CACHE = {}


def _host_prep(inputs):
    w1s = np.asarray(inputs["w1s"], np.float32)
    w1v = np.asarray(inputs["w1v"], np.float32)
    A = np.zeros((16, 10, 10), np.float32)
    A[:, 0, 9] = w1s[0, :, 0, 0]
    s3 = 1.0 / math.sqrt(3.0)
    for v in range(3):
        for m in range(3):
            for u in range(3):
                A[:, 1 + 3 * v + m, 3 * u + m] = w1v[0, :, u, v] * s3
    w1pt = np.ascontiguousarray(np.transpose(A, (2, 0, 1)).reshape(10, 160))
    d_inT = np.ascontiguousarray(np.asarray(inputs["D_in"], np.float32).T)  # (10,4096)
    # packed for PE row-groups: k-tile k at rows 32*(k%4)..+10, cols 128*(k//4)..+128
    dinp = np.zeros((128, 1024), np.float32)
    for k in range(32):
        dinp[32 * (k % 4) : 32 * (k % 4) + 10, 128 * (k // 4) : 128 * (k // 4) + 128] = \
            d_inT[:, k * 128 : (k + 1) * 128]
    icoT = np.ascontiguousarray(np.asarray(inputs["ico_wigners"], np.float32).T)  # (455,60)
    w2c = {}
    for l in range(LMAX + 1):
        d = DS[l]
        w2 = np.asarray(inputs[f"w2_{l}"], np.float32)  # (c,g,u,v)
        w2c[l] = np.ascontiguousarray(
            np.transpose(w2, (2, 0, 3, 1)).reshape(16 * d, 64 * d) / math.sqrt(16 * d)
        )
    return w1pt, dinp, icoT, w2c


def _vm_tiles(l):
    d2 = DS[l] * DS[l]
    out = []
    r0 = 0
    while r0 < d2:
        out.append((r0, min(128, d2 - r0)))
        r0 += 128
    return out


def _uc_tiles(l):
    d = DS[l]
    out = []
    u0 = 0
    while u0 < d:
        nu = min(8, d - u0)
        out.append((u0, nu))
        u0 += nu
    return out


def _n_chunks(l):
    n = 64 * DS[l]
    if n <= 512:
        return [(0, n)]
    h = n // 2
    return [(0, h), (h, n - h)]


def _build(trace_sim=False, niter=1):
    import concourse.bacc as bacc
    import concourse.mybir as mybir
    import concourse.tile as tile

    F32R = mybir.dt.float32r
    F32 = mybir.dt.float32
    RELU = mybir.ActivationFunctionType.Relu
    COPY = mybir.ActivationFunctionType.Copy

    nc = bacc.Bacc("TRN2", target_bir_lowering=False, debug=False)

    x_in = nc.dram_tensor("x_in", (BHC, J455), F32R, kind="ExternalInput").ap()
    trajT_in = nc.dram_tensor("trajT_in", (10, R), F32R, kind="ExternalInput").ap()
    w1pt_in = nc.dram_tensor("w1pt_in", (10, 160), F32R, kind="ExternalInput").ap()
    dinp_in = nc.dram_tensor("dinp_in", (128, 1024), F32R, kind="ExternalInput").ap()
    doutw_in = nc.dram_tensor("doutw_in", (G, J455), F32R, kind="ExternalInput").ap()
    icoT_in = nc.dram_tensor("icoT_in", (J455, 60), F32R, kind="ExternalInput").ap()
    w2c_in = {
        l: nc.dram_tensor(f"w2c{l}_in", (16 * DS[l], 64 * DS[l]), F32R, kind="ExternalInput").ap()
        for l in range(LMAX + 1)
    }
    ident_in = nc.dram_tensor("ident_in", (128, 128), F32R, kind="ExternalInput").ap()
    out_xgT = nc.dram_tensor("out_xgT", (60, BHC), F32, kind="ExternalOutput").ap()
    out_trajT = nc.dram_tensor("out_trajT", (60, 64 * R), F32, kind="ExternalOutput").ap()
    scr1 = nc.dram_tensor("scr1", (J455, NCOL), F32R).ap()
    scr2 = nc.dram_tensor("scr2", (J455, 64 * R), F32R).ap()

    JC = [(0, 128), (128, 128), (256, 128), (384, 71)]  # j-chunks of 455
    JT = [(0, 128), (128, 128), (256, 128), (384, 71)]  # j-tiles for x path

    with tile.TileContext(nc, trace_sim=trace_sim) as tc:
        with (
            tc.tile_pool(name="const", bufs=1) as cpool,
            tc.tile_pool(name="sig", bufs=1) as sigpool,
            tc.tile_pool(name="work", bufs=1) as wpool,
            tc.tile_pool(name="ps", bufs=1, space="PSUM") as pspool,
        ):
            # ---------- constants (outside the repeat loop) ----------
            w1pt_sb = cpool.tile([10, 160], F32R, name="w1pt_sb")
            nc.sync.dma_start(w1pt_sb[:], w1pt_in[:])
            dinp_sb = cpool.tile([128, 1024], F32R, name="dinp_sb")
            nc.sync.dma_start(dinp_sb[:], dinp_in[:])
            trajT_sb = cpool.tile([10, R], F32R, name="trajT_sb")
            nc.sync.dma_start(trajT_sb[:], trajT_in[:])
            doutw_sb = cpool.tile([128, 32 * J455], F32R, name="doutw_sb")
            for k in range(32):
                nc.sync.dma_start(
                    doutw_sb[:, k * J455 : (k + 1) * J455],
                    doutw_in[k * 128 : (k + 1) * 128, :],
                )
            ident = cpool.tile([128, 128], F32R, name="ident")
            nc.sync.dma_start(ident[:], ident_in[:])
            icoX_sb = {}
            for jt, (j0, jn) in enumerate(JT):
                t = cpool.tile([128, 60], F32R, name=f"icoX{jt}", tag=f"icoX{jt}")
                nc.sync.dma_start(t[0:jn, :], icoT_in[j0 : j0 + jn, :])
                icoX_sb[jt] = t
            icoD_sb = {}
            for l in range(LMAX + 1):
                for ki, (r0, nr) in enumerate(_vm_tiles(l)):
                    t = cpool.tile([128, 60], F32R, name=f"icoD{l}_{ki}", tag=f"icoD{l}_{ki}")
                    nc.sync.dma_start(t[0:nr, :], icoT_in[OFF[l] + r0 : OFF[l] + r0 + nr, :])
                    icoD_sb[(l, ki)] = t
            hT_sb = cpool.tile([128, NCOL], F32R, name="hT_sb")

            def emit_hT():
                for half in range(2):
                    ph = pspool.tile([128, 512], F32, name=f"ps_h{half}", tag="ps", bufs=6)
                    for cc in range(8):
                        c = half * 8 + cc
                        nc.tensor.matmul(
                            ph[0:10, cc * R : (cc + 1) * R],
                            w1pt_sb[:, c * 10 : (c + 1) * 10],
                            trajT_sb[:],
                            start=True, stop=True,
                        )
                    nc.vector.tensor_copy(
                        hT_sb[0:10, half * 512 : (half + 1) * 512], ph[0:10, :]
                    )
                for base in (32, 64, 96):
                    nc.sync.dma_start(hT_sb[base : base + 10, :], hT_sb[0:10, :])

            def emit_x_chunk(ch):
                xT = {}
                for jt, (j0, jn) in enumerate(JT):
                    xT[jt] = wpool.tile([128, 512], F32R, name=f"xT{ch}_{jt}", tag=f"xT{jt}", bufs=2)
                for i in range(4):
                    xn = wpool.tile([128, J455], F32R, name=f"xn{ch}_{i}", tag="xnat", bufs=4)
                    nc.sync.dma_start(xn[:], x_in[ch * 512 + i * 128 : ch * 512 + (i + 1) * 128, :])
                    for jt, (j0, jn) in enumerate(JT):
                        pt = pspool.tile([128, 128], F32R, name=f"pt{ch}_{i}_{jt}", tag="psT", bufs=2)
                        nc.tensor.transpose(pt[0:jn, :], xn[:, j0 : j0 + jn], ident[:])
                        nc.vector.tensor_copy(
                            xT[jt][0:jn, i * 128 : (i + 1) * 128], pt[0:jn, :]
                        )
                px = pspool.tile([128, 512], F32, name=f"px{ch}", tag="ps", bufs=6)
                for jt, (j0, jn) in enumerate(JT):
                    nc.tensor.matmul(
                        px[0:60, :], icoX_sb[jt][0:jn, :], xT[jt][0:jn, :],
                        start=(jt == 0), stop=(jt == 3),
                    )
                xo = wpool.tile([60, 512], F32, name=f"xo{ch}", tag="ost", bufs=2)
                nc.scalar.activation(xo[:], px[0:60, :], COPY)
                nc.sync.dma_start(out_xgT[:, ch * 512 : (ch + 1) * 512], xo[:])

            def emit_quarter(q):
                q0 = q * QW
                sig_sb = {}
                for k in range(32):
                    base = 32 * (k % 4)
                    pa = pspool.tile([128, 512], F32, name=f"pa{q}_{k}", tag="ps", bufs=6)
                    nc.tensor.matmul(
                        pa[:, 0:QW],
                        dinp_sb[base : base + 10, 128 * (k // 4) : 128 * (k // 4) + 128],
                        hT_sb[base : base + 10, q0 : q0 + QW],
                        start=True, stop=True,
                        tile_position=(base, 0),
                    )
                    sg = sigpool.tile([128, QW], F32R, name=f"sig{q}_{k}", tag=f"sig{k}", bufs=1)
                    if k % 2 == 0:
                        nc.scalar.activation(sg[:], pa[:, 0:QW], RELU)
                    else:
                        nc.vector.tensor_scalar_max(sg[:], pa[:, 0:QW], 0.0)
                    sig_sb[k] = sg
                for jc, (j0, jn) in enumerate(JC):
                    pb = pspool.tile([128, 512], F32, name=f"pb{q}_{jc}", tag="ps", bufs=6)
                    for k in range(32):
                        nc.tensor.matmul(
                            pb[0:jn, 0:QW],
                            doutw_sb[:, k * J455 + j0 : k * J455 + j0 + jn],
                            sig_sb[k][:],
                            start=(k == 0), stop=(k == 31),
                        )
                    h2s = wpool.tile([128, QW], F32R, name=f"h2s{q}_{jc}", tag="h2st", bufs=3)
                    nc.vector.tensor_copy(h2s[0:jn, :], pb[0:jn, 0:QW])
                    nc.sync.dma_start(scr1[j0 : j0 + jn, q0 : q0 + QW], h2s[0:jn, :])

            def emit_cl(l):
                d = DS[l]
                uc = _uc_tiles(l)
                X = {}
                for ki, (u0, nu) in enumerate(uc):
                    xt = wpool.tile([128, 64 * 13], F32R, name=f"X{l}_{ki}", tag="X", bufs=3)
                    X[ki] = xt
                    for uu in range(nu):
                        u = u0 + uu
                        src = scr1[OFF[l] + u * d : OFF[l] + (u + 1) * d, :].rearrange(
                            "m (c bt) -> c m bt", c=16
                        )
                        nc.sync.dma_start(xt[uu * 16 : (uu + 1) * 16, 0 : 64 * d], src)
                w2 = {}
                for ki, (u0, nu) in enumerate(uc):
                    wt = wpool.tile([128, 64 * 13], F32R, name=f"w2t{l}_{ki}", tag="w2c", bufs=3)
                    nc.sync.dma_start(
                        wt[0 : 16 * nu, 0 : 64 * d], w2c_in[l][16 * u0 : 16 * (u0 + nu), :]
                    )
                    w2[ki] = wt
                mtot = 64 * d
                stages = []
                mc0 = 0
                while mc0 < mtot:
                    msz = min(128, mtot - mc0)
                    st = wpool.tile([128, 64 * 13], F32R, name=f"st{l}_{mc0}", tag="stage", bufs=8)
                    pc = pspool.tile([128, 512], F32, name=f"pc{l}_{mc0}", tag="ps", bufs=6)
                    for (n0, nn) in _n_chunks(l):
                        for ki in range(len(uc)):
                            nk = 16 * uc[ki][1]
                            nc.tensor.matmul(
                                pc[0:msz, 0:nn],
                                w2[ki][0:nk, mc0 : mc0 + msz],
                                X[ki][0:nk, n0 : n0 + nn],
                                start=(ki == 0), stop=(ki == len(uc) - 1),
                            )
                        nc.vector.tensor_copy(st[0:msz, n0 : n0 + nn], pc[0:msz, 0:nn])
                    stages.append((mc0, msz, st))
                    mc0 += msz
                for v in range(d):
                    mc0, msz, st = stages[(v * 64) // 128]
                    loc = v * 64 - mc0
                    dst = scr2[OFF[l] + v * d : OFF[l] + (v + 1) * d, :].rearrange(
                        "m (g bt) -> g m bt", g=64
                    )
                    nc.sync.dma_start(dst, st[loc : loc + 64, 0 : 64 * d])

            def emit_d(nch):
                pd = pspool.tile([128, 512], F32, name=f"pd{nch}", tag="ps", bufs=6)
                groups = [(l, ki, r0, nr) for l in range(LMAX + 1)
                          for ki, (r0, nr) in enumerate(_vm_tiles(l))]
                for gi, (l, ki, r0, nr) in enumerate(groups):
                    tt = wpool.tile([128, 512], F32R, name=f"tt{nch}_{gi}", tag="tT", bufs=3)
                    nc.sync.dma_start(
                        tt[0:nr, :],
                        scr2[OFF[l] + r0 : OFF[l] + r0 + nr, nch * 512 : (nch + 1) * 512],
                    )
                    nc.tensor.matmul(
                        pd[0:60, :], icoD_sb[(l, ki)][0:nr, :], tt[0:nr, :],
                        start=(gi == 0), stop=(gi == len(groups) - 1),
                    )
                to = wpool.tile([60, 512], F32, name=f"to{nch}", tag="ost", bufs=2)
                nc.scalar.activation(to[:], pd[0:60, :], COPY)
                nc.sync.dma_start(out_trajT[:, nch * 512 : (nch + 1) * 512], to[:])

            def emit_body():
                emit_hT()
                for q in range(NQ):
                    emit_x_chunk(q)
                    emit_quarter(q)
                for l in range(LMAX, -1, -1):
                    emit_cl(l)
                for nch in range(8):
                    emit_d(nch)

            if niter > 1:
                with tc.For_i(0, niter, 1):
                    emit_body()
            else:
                emit_body()

    nc.compile()
    return nc


def _get_nc():
    if "nc" not in _CACHE:
        _CACHE["nc"] = _build()
    return _CACHE["nc"]


def make_in_maps(inputs):
    w1pt, dinp, icoT, w2c = _host_prep(inputs)
    x = np.asarray(inputs["x"], np.float32)
    traj = np.asarray(inputs["trajectory"], np.float32)
    doutw = np.ascontiguousarray(np.asarray(inputs["D_outw"], np.float32))
    shared = {
        "w1pt_in": w1pt,
        "dinp_in": dinp,
        "doutw_in": doutw,
        "icoT_in": icoT,
        "ident_in": np.eye(128, dtype=np.float32),
    }
    for l in range(LMAX + 1):
        shared[f"w2c{l}_in"] = w2c[l]
    in_maps = []
    for i in range(NCORES):
        xs = np.ascontiguousarray(x[i * B_LOC : (i + 1) * B_LOC].reshape(BHC, J455))
        ts = np.ascontiguousarray(traj[i * B_LOC : (i + 1) * B_LOC].reshape(R, 10).T)
        in_maps.append(dict(shared, x_in=xs, trajT_in=ts))
    return in_maps


def assemble(results):
    xg = np.zeros((32, 4, 128, 60), np.float32)
    trj = np.zeros((32, 16, 64, 60), np.float32)
    for i, res in enumerate(results):
        xg[i * B_LOC : (i + 1) * B_LOC] = (
            res["out_xgT"].T.reshape(B_LOC, 4, 128, 60)
        )
        trj[i * B_LOC : (i + 1) * B_LOC] = (
            res["out_trajT"].reshape(60, 64, B_LOC, 16).transpose(2, 3, 1, 0)
        )
    return xg, trj


def kernel(**inputs):
    from concourse.bass_utils import run_bass_kernel_spmd

    nc = _get_nc()
    in_maps = make_in_maps(inputs)
    res = run_bass_kernel_spmd(nc, in_maps, core_ids=list(range(NCORES)), trace=False)
    _CACHE["last_result"] = res
    return assemble(res.results)
